# revision 1
# baseline (speedup 1.0000x reference)
"""GAT (2-layer, 3-head) forward on 8 Trainium2 NeuronCores.

Sharding: nodes split 8 ways; each core owns 12544 padded destination nodes
and all their incoming edges (1D graph partition per the spec hint). A
channel-major node table (h | a_src | a_dst, 15 ch) is replicated into SBUF
as 4 quarters x 2 copies across the 8 GPSIMD 16-partition groups; per-edge
features stream out via ap_gather with per-group index streams laid out in
dst-canonical order with K=8 slots per (dst, group) (A/B copy balancing;
rare per-(dst,quarter) overflow rows are folded back with a second small
gather). Edge softmax runs densely on DVE/ACT over the slot grid; the
slot-window reduction uses an avg-pool (the num/den ratio is scale
invariant); cross-group partial sums combine with one PE matmul. Three NEFF
launches: (A) table build (x @ W1aug on PE), (B) edge layer 1 + layer-2
table build, (C) edge layer 2 + head-mean + log_softmax. Tables are
all-gathered between launches through the host.
"""
import sys
import types

sys.path.insert(0, "/opt/trn_rl_repo")
import numpy as np

N_NODES = 100000
IN_DIM = 256
HID = 3
HEADS = 3
NCLS = 3
NEG = 0.2
EPS = 1e-16

NQ = 4
QREAL = 25000
QN = 25088
NPAD = NQ * QN          # 100352
NCORE = 8
CN = NPAD // NCORE      # 12544
K = 8
DCHUNK = 224
NCHUNK = CN // DCHUNK   # 56
RPAD = CN + 2 * DCHUNK  # 12992
GCHUNK = RPAD // DCHUNK  # 57
SLOTS = RPAD * K
SCHUNK = DCHUNK * K     # 1792
SENT = QREAL
ZCOL = RPAD - 1
CH = 15
FCH = 8
BIG_NEG = -30000.0

LAST_STATS = {}


def _install_ntff_hook_module():
    if "antenv.axon_hooks" in sys.modules:
        return
    mod = types.ModuleType("antenv.axon_hooks")
    state = {"hook": None, "tried": False}

    def set_axon_ntff_profile_hook(hook):
        state["hook"] = hook

    def get_axon_ntff_profile_hook():
        if state["hook"] is None and not state["tried"]:
            state["tried"] = True
            try:
                from trn_agent_boot.trn_boot import _ntff_profile_via_ctypes

                state["hook"] = _ntff_profile_via_ctypes("/opt/axon/libaxon_pjrt.so")
            except Exception:
                state["hook"] = None
        return state["hook"]

    mod.set_axon_ntff_profile_hook = set_axon_ntff_profile_hook
    mod.get_axon_ntff_profile_hook = get_axon_ntff_profile_hook
    sys.modules["antenv.axon_hooks"] = mod


_install_ntff_hook_module()

import concourse.bass as bass
import concourse.mybir as mybir
import concourse.tile as tile
from concourse.bass_utils import run_bass_kernel_spmd
from concourse.library_overlay import lower_extended_insts
from concourse import library_config

F32 = mybir.dt.float32
I16 = mybir.dt.int16
ALU = mybir.AluOpType
ACT = mybir.ActivationFunctionType


def _split_wide_waits(nc):
    """Walrus here caps sync-wait commands per instruction; hoist excess waits
    onto preceding same-engine nofuse NOPs (engines execute in order)."""
    for fn in nc.m.functions:
        for bb in fn.blocks:
            new_insts = []
            for inst in bb.instructions:
                keep = 0 if isinstance(inst, mybir.InstDrain) else 1
                si = inst.sync_info
                if si is not None and si.on_wait is not None and len(si.on_wait) > keep:
                    waits = list(si.on_wait)
                    head, rest = (waits[:-keep], waits[-keep:]) if keep else (waits, [])
                    while head:
                        chunk, head = head[:1], head[1:]
                        nop = mybir.InstNoOp(name=f"I-{nc.next_id()}", ins=[], outs=[])
                        nop.engine = inst.engine
                        nop.bass_nofuse = True
                        nop.sync_info = mybir.SyncInfo(on_wait=chunk, on_update=[])
                        nc.register_instruction(nop, overwrite=True)
                        new_insts.append(nop)
                    inst.sync_info = mybir.SyncInfo(
                        on_wait=rest, on_update=list(si.on_update or [])
                    )
                new_insts.append(inst)
            bb.instructions.clear()
            for i in new_insts:
                bb.add_instruction(i)


def _run(nc, in_maps, trace=False):
    lower_extended_insts(nc)
    _split_wide_waits(nc)
    return run_bass_kernel_spmd(nc, in_maps, core_ids=list(range(NCORE)), trace=trace)


# ---------------------------------------------------------------- launch A
def _build_phase_a():
    nc = bass.Bass("TRN2")
    xT_d = nc.dram_tensor("xT", [IN_DIM, CN], F32, kind="ExternalInput")
    w1_d = nc.dram_tensor("w1", [IN_DIM, HEADS * HID], F32, kind="ExternalInput")
    w1t_d = nc.dram_tensor("w1t", [HEADS * HID, IN_DIM], F32, kind="ExternalInput")
    attw1_d = nc.dram_tensor("attw1", [HEADS * HID, 6], F32, kind="ExternalInput")
    tab_d = nc.dram_tensor("tab", [CH, CN], F32, kind="ExternalOutput")

    with tile.TileContext(nc) as tc:
        with (
            tc.tile_pool(name="const", bufs=1) as cpool,
            tc.tile_pool(name="io", bufs=3) as iopool,
            tc.tile_pool(name="ps", bufs=2, space="PSUM") as pspool,
        ):
            w1aug = cpool.tile([128, 2 * CH], F32)
            w1t = cpool.tile([HEADS * HID, IN_DIM], F32)
            attw1 = cpool.tile([HEADS * HID, 6], F32)
            nc.sync.dma_start(w1t[:], w1t_d[:])
            nc.sync.dma_start(attw1[:], attw1_d[:])
            for k in range(2):
                nc.sync.dma_start(
                    w1aug[:, CH * k:CH * k + 9], w1_d[128 * k:128 * (k + 1), :]
                )
                vps = pspool.tile([128, 6], F32, tag="vps")
                nc.tensor.matmul(
                    out=vps[:],
                    lhsT=w1t[:, 128 * k:128 * (k + 1)],
                    rhs=attw1[:],
                    start=True,
                    stop=True,
                )
                nc.vector.tensor_copy(out=w1aug[:, CH * k + 9:CH * k + 15], in_=vps[:])
            for c in range(NCHUNK):
                cols = slice(DCHUNK * c, DCHUNK * (c + 1))
                ps = pspool.tile([CH, DCHUNK], F32, tag="ps")
                for k in range(2):
                    xc = iopool.tile([128, DCHUNK], F32, tag="xc")
                    nc.sync.dma_start(xc[:], xT_d[128 * k:128 * (k + 1), cols])
                    nc.tensor.matmul(
                        out=ps[:],
                        lhsT=w1aug[:, CH * k:CH * (k + 1)],
                        rhs=xc[:],
                        start=(k == 0),
                        stop=(k == 1),
                    )
                ob = iopool.tile([CH, DCHUNK], F32, tag="ob")
                nc.vector.tensor_copy(out=ob[:], in_=ps[:])
                nc.sync.dma_start(tab_d[:, cols], ob[:])
    return nc


# ---------------------------------------------------------------- launch B/C
def _build_edge(final):
    nc = bass.Bass("TRN2")
    tab_d = nc.dram_tensor("tabf", [CH, NPAD], F32, kind="ExternalInput")
    idx_d = nc.dram_tensor("idxs", [128, SLOTS // 16], I16, kind="ExternalInput")
    ov_d = nc.dram_tensor("ovidx", [128, CN // 16], I16, kind="ExternalInput")
    adrep_d = nc.dram_tensor("adrep", [24, RPAD], F32, kind="ExternalInput")
    lhsn_d = nc.dram_tensor("lhsn", [128, 9], F32, kind="ExternalInput")
    lhsd_d = nc.dram_tensor("lhsd", [128, 9], F32, kind="ExternalInput")
    bias_d = nc.dram_tensor("biasv", [9, 1], F32, kind="ExternalInput")
    if final:
        meanw_d = nc.dram_tensor("meanw", [9, NCLS], F32, kind="ExternalInput")
        ones3_d = nc.dram_tensor("ones3", [NCLS, 1], F32, kind="ExternalInput")
        ones1_d = nc.dram_tensor("ones1", [1, NCLS], F32, kind="ExternalInput")
        out_d = nc.dram_tensor("outp", [NCLS, CN], F32, kind="ExternalOutput")
    else:
        w2t_d = nc.dram_tensor("w2t", [9, 9], F32, kind="ExternalInput")
        w2_d = nc.dram_tensor("w2", [9, 9], F32, kind="ExternalInput")
        attw2_d = nc.dram_tensor("attw2", [9, 6], F32, kind="ExternalInput")
        tab2_d = nc.dram_tensor("tab2", [CH, CN], F32, kind="ExternalOutput")

    with tile.TileContext(nc) as tc:
        with (
            tc.tile_pool(name="big", bufs=1) as bigpool,
            tc.tile_pool(name="io", bufs=2) as iopool,
            tc.tile_pool(name="gp", bufs=3) as gpool,
            tc.tile_pool(name="sm", bufs=8) as smpool,
            tc.tile_pool(name="ps", bufs=2, space="PSUM") as pspool,
            tc.tile_pool(name="psf", bufs=1, space="PSUM") as psfpool,
        ):
            table = bigpool.tile([128, QN], F32)
            partials = bigpool.tile([128, RPAD], F32)
            nc.vector.memset(partials[:], 0.0)
            for g in range(8):
                q = g % 4
                nc.sync.dma_start(
                    table[16 * g:16 * g + CH, :], tab_d[:, QN * q:QN * (q + 1)]
                )
            lhsn = bigpool.tile([128, 9], F32)
            nc.sync.dma_start(lhsn[:], lhsn_d[:])
            lhsd = bigpool.tile([128, 9], F32)
            nc.sync.dma_start(lhsd[:], lhsd_d[:])
            biasv = bigpool.tile([9, 1], F32)
            nc.sync.dma_start(biasv[:], bias_d[:])
            if final:
                meanw = bigpool.tile([9, NCLS], F32)
                ones3 = bigpool.tile([NCLS, 1], F32)
                ones1 = bigpool.tile([1, NCLS], F32)
                nc.sync.dma_start(meanw[:], meanw_d[:])
                nc.sync.dma_start(ones3[:], ones3_d[:])
                nc.sync.dma_start(ones1[:], ones1_d[:])
            else:
                w2aug = bigpool.tile([9, CH], F32)
                w2t = smpool.tile([9, 9], F32, tag="sm")
                attw2 = smpool.tile([9, 6], F32, tag="sm")
                nc.sync.dma_start(w2t[:], w2t_d[:])
                nc.sync.dma_start(attw2[:], attw2_d[:])
                nc.sync.dma_start(w2aug[:, 0:9], w2_d[:])
                v2ps = psfpool.tile([9, 6], F32, tag="v2")
                nc.tensor.matmul(
                    out=v2ps[:], lhsT=w2t[:], rhs=attw2[:], start=True, stop=True
                )
                nc.vector.tensor_copy(out=w2aug[:, 9:15], in_=v2ps[:])

            tab_in = table[:].rearrange("p (n d) -> p n d", d=1)
            w9 = bigpool.tile([128, SCHUNK], F32)
            nc.vector.memset(w9[:], 1.0)
            nc.gpsimd.load_library(library_config.ap_gather)

            # ---- gather + per-slot softmax weights + messages ----
            for c in range(GCHUNK):
                scol = slice(SCHUNK // 16 * c, SCHUNK // 16 * (c + 1))
                dcol = slice(DCHUNK * c, DCHUNK * (c + 1))
                idxc = iopool.tile([128, SCHUNK // 16], I16, tag="idxc")
                nc.sync.dma_start(idxc[:], idx_d[:, scol])
                g_t = gpool.tile([128, SCHUNK], F32, tag="g")
                nc.gpsimd.ap_gather(
                    out_ap=g_t[:].rearrange("p (n d) -> p n d", d=1),
                    in_ap=tab_in,
                    idxs_ap=idxc[:],
                    channels=128,
                    num_elems=QN,
                    d=1,
                    num_idxs=SCHUNK,
                )
                a24 = iopool.tile([24, DCHUNK], F32, tag="a24")
                nc.sync.dma_start(a24[:], adrep_d[:, dcol])
                wt = iopool.tile([24, SCHUNK], F32, tag="wt")
                for g in range(8):
                    nc.sync.dma_start(
                        wt[3 * g:3 * g + 3, :], g_t[16 * g + 9:16 * g + 12, :]
                    )
                wt3 = wt[:].rearrange("p (n j) -> p n j", j=K)
                nc.vector.tensor_tensor(
                    out=wt3, in0=wt3,
                    in1=a24[:].to_broadcast([24, DCHUNK, K]), op=ALU.add,
                )
                # leaky relu: (x * 0.2) max x, then exp
                nc.vector.scalar_tensor_tensor(
                    out=wt[:], in0=wt[:], scalar=NEG, in1=wt[:],
                    op0=ALU.mult, op1=ALU.max,
                )
                nc.scalar.activation(out=wt[:], in_=wt[:], func=ACT.Exp)
                for h in range(3):
                    nc.sync.dma_start(g_t[9 + h::16, :], wt[h::3, :])
                    for ch3 in range(3):
                        nc.sync.dma_start(w9[3 * h + ch3::16, :], wt[h::3, :])
                nc.vector.tensor_tensor(
                    out=g_t[:], in0=g_t[:], in1=w9[:], op=ALU.mult
                )
                nc.vector.tensor_reduce(
                    out=partials[:, dcol],
                    in_=g_t[:].rearrange("p (n j) -> p n j", j=K),
                    axis=mybir.AxisListType.X,
                    op=ALU.add,
                )

            # ---- overflow fold + cross-group combine + per-node math ----
            par_in = partials[:].rearrange("p (n d) -> p n d", d=1)
            for c in range(NCHUNK):
                dcol = slice(DCHUNK * c, DCHUNK * (c + 1))
                ovc = iopool.tile([128, DCHUNK // 16], I16, tag="ovc")
                nc.sync.dma_start(
                    ovc[:], ov_d[:, DCHUNK // 16 * c:DCHUNK // 16 * (c + 1)]
                )
                foldt = iopool.tile([128, DCHUNK], F32, tag="fold")
                nc.gpsimd.ap_gather(
                    out_ap=foldt[:].rearrange("p (n d) -> p n d", d=1),
                    in_ap=par_in,
                    idxs_ap=ovc[:],
                    channels=128,
                    num_elems=RPAD,
                    d=1,
                    num_idxs=DCHUNK,
                )
                fold = foldt[:, :]
                ndn_ps = pspool.tile([9, DCHUNK], F32, tag="ndn")
                ndd_ps = pspool.tile([9, DCHUNK], F32, tag="ndd")
                nc.tensor.matmul(
                    out=ndn_ps[:], lhsT=lhsn[:], rhs=partials[:, dcol],
                    start=True, stop=False,
                )
                nc.tensor.matmul(
                    out=ndn_ps[:], lhsT=lhsn[:], rhs=fold, start=False, stop=True
                )
                nc.tensor.matmul(
                    out=ndd_ps[:], lhsT=lhsd[:], rhs=partials[:, dcol],
                    start=True, stop=False,
                )
                nc.tensor.matmul(
                    out=ndd_ps[:], lhsT=lhsd[:], rhs=fold, start=False, stop=True
                )
                ndn = smpool.tile([9, DCHUNK], F32, tag="sm")
                nc.vector.tensor_copy(out=ndn[:], in_=ndn_ps[:])
                rden9 = smpool.tile([9, DCHUNK], F32, tag="sm")
                nc.vector.tensor_scalar_add(
                    out=rden9[:], in0=ndd_ps[:], scalar1=EPS
                )
                nc.vector.reciprocal(out=rden9[:], in_=rden9[:])
                hagg = smpool.tile([9, DCHUNK], F32, tag="sm")
                nc.vector.tensor_tensor(
                    out=hagg[:], in0=ndn[:], in1=rden9[:], op=ALU.mult
                )
                if not final:
                    nc.vector.tensor_tensor(
                        out=hagg[:], in0=hagg[:],
                        in1=biasv[:].to_broadcast([9, DCHUNK]), op=ALU.add,
                    )
                    t1 = smpool.tile([9, DCHUNK], F32, tag="sm")
                    nc.vector.tensor_scalar_min(out=t1[:], in0=hagg[:], scalar1=0.0)
                    nc.scalar.activation(out=t1[:], in_=t1[:], func=ACT.Exp)
                    # elu = relu(x) + exp(min(x,0)) - 1
                    nc.vector.tensor_scalar_max(out=hagg[:], in0=hagg[:], scalar1=0.0)
                    nc.vector.tensor_tensor(
                        out=hagg[:], in0=hagg[:], in1=t1[:], op=ALU.add
                    )
                    nc.vector.tensor_scalar_add(out=hagg[:], in0=hagg[:], scalar1=-1.0)
                    t2ps = psfpool.tile([CH, DCHUNK], F32, tag="t2")
                    nc.tensor.matmul(
                        out=t2ps[:], lhsT=w2aug[:], rhs=hagg[:], start=True, stop=True
                    )
                    t2sb = smpool.tile([CH, DCHUNK], F32, tag="sm")
                    nc.vector.tensor_copy(out=t2sb[:], in_=t2ps[:])
                    nc.sync.dma_start(tab2_d[:, dcol], t2sb[:])
                else:
                    zps = psfpool.tile([NCLS, DCHUNK], F32, tag="z")
                    nc.tensor.matmul(
                        out=zps[:], lhsT=meanw[:], rhs=hagg[:], start=True, stop=True
                    )
                    z = smpool.tile([NCLS, DCHUNK], F32, tag="sm")
                    nc.vector.tensor_copy(out=z[:], in_=zps[:])
                    nc.vector.tensor_tensor(
                        out=z[:], in0=z[:],
                        in1=biasv[0:3, :].to_broadcast([NCLS, DCHUNK]), op=ALU.add,
                    )
                    ez = smpool.tile([NCLS, DCHUNK], F32, tag="sm")
                    nc.scalar.activation(out=ez[:], in_=z[:], func=ACT.Exp)
                    sps = psfpool.tile([1, DCHUNK], F32, tag="s")
                    nc.tensor.matmul(
                        out=sps[:], lhsT=ones3[:], rhs=ez[:], start=True, stop=True
                    )
                    s = smpool.tile([1, DCHUNK], F32, tag="sm")
                    nc.vector.tensor_copy(out=s[:], in_=sps[:])
                    nc.scalar.activation(out=s[:], in_=s[:], func=ACT.Ln)
                    l3ps = psfpool.tile([NCLS, DCHUNK], F32, tag="l3")
                    nc.tensor.matmul(
                        out=l3ps[:], lhsT=ones1[:], rhs=s[:], start=True, stop=True
                    )
                    l3 = smpool.tile([NCLS, DCHUNK], F32, tag="sm")
                    nc.vector.tensor_copy(out=l3[:], in_=l3ps[:])
                    zm = smpool.tile([NCLS, DCHUNK], F32, tag="sm")
                    nc.vector.tensor_tensor(
                        out=zm[:], in0=z[:], in1=l3[:], op=ALU.subtract
                    )
                    nc.sync.dma_start(out_d[:, dcol], zm[:])
    return nc


# ---------------------------------------------------------------- host side
def _relabel(n):
    q = n // QREAL
    return q * QN + n % QREAL


def _wrap_chunked(stream, chunk):
    """[G, S] streams -> [16G, S//16] ap_gather idx layout, wrapped per chunk."""
    g, s = stream.shape
    nch = s // chunk
    w = stream.reshape(g, nch, chunk // 16, 16)
    w = w.transpose(0, 3, 1, 2)
    return np.ascontiguousarray(w.reshape(g * 16, s // 16))


def _pack_edges(src, dst):
    srcN = _relabel(src.astype(np.int64))
    dstN = _relabel(dst.astype(np.int64))
    core = dstN // CN
    dloc = dstN % CN
    q = srcN // QN
    sloc = (srcN % QN).astype(np.int16)

    key = (core * CN + dloc) * 4 + q
    order = np.argsort(key, kind="stable")
    ks = key[order]
    grp_start = np.r_[0, np.flatnonzero(np.diff(ks)) + 1]
    sizes = np.diff(np.r_[grp_start, len(ks)])
    rank = np.arange(len(ks)) - np.repeat(grp_start, sizes)

    co, dl, qo, sl = core[order], dloc[order], q[order], sloc[order]

    streams = np.full((NCORE, 8, SLOTS), SENT, dtype=np.int16)
    ovidx = np.full((NCORE, CN), ZCOL, dtype=np.int16)
    ovdst = [[] for _ in range(NCORE)]

    main = rank < 16
    gmain = qo[main] + 4 * (rank[main] & 1)
    pos = dl[main] * K + (rank[main] >> 1)
    streams[co[main], gmain, pos] = sl[main]

    for i in np.flatnonzero(~main):
        c, d, qq, s_, r = co[i], dl[i], qo[i], sl[i], rank[i]
        if ovidx[c, d] == ZCOL:
            row = CN + len(ovdst[c])
            assert row < RPAD - 1, "overflow area exhausted"
            ovidx[c, d] = row
            ovdst[c].append(int(d))
        rr = r - 16
        assert rr < 16, "per-(dst,quarter) capacity exceeded"
        g = qq + 4 * (rr & 1)
        streams[c, g, int(ovidx[c, d]) * K + (rr >> 1)] = s_
    return streams, ovidx, ovdst


def kernel(x, edge_index, W1, att_src1, att_dst1, b1, W2, att_src2, att_dst2, b2):
    import os as _os
    import time as _time

    x = np.asarray(x, np.float32)
    W1 = np.asarray(W1, np.float32)
    W2 = np.asarray(W2, np.float32)
    b1v = np.asarray(b1, np.float32)
    b2v = np.asarray(b2, np.float32)

    loops = np.arange(N_NODES, dtype=np.int64)
    src = np.concatenate([np.asarray(edge_index[0], np.int64), loops])
    dst = np.concatenate([np.asarray(edge_index[1], np.int64), loops])
    streams, ovidx, ovdst = _pack_edges(src, dst)

    xP = np.zeros((NPAD, IN_DIM), np.float32)
    xP[_relabel(np.arange(N_NODES))] = x
    xT = np.ascontiguousarray(xP.T)

    def attw(att_s, att_d):
        a = np.zeros((HEADS * HID, 6), np.float32)
        for h in range(HEADS):
            for cc in range(3):
                a[3 * h + cc, h] = np.asarray(att_s, np.float32)[h, cc]
                a[3 * h + cc, 3 + h] = np.asarray(att_d, np.float32)[h, cc]
        return a

    attw1 = attw(att_src1, att_dst1)
    attw2 = attw(att_src2, att_dst2)

    lhsn = np.zeros((128, 9), np.float32)
    lhsd = np.zeros((128, 9), np.float32)
    for p in range(128):
        j = p % 16
        if j < 9:
            lhsn[p, j] = 1.0
        elif j < 12:
            for cc in range(3):
                lhsd[p, 3 * (j - 9) + cc] = 1.0
    meanw = np.zeros((9, NCLS), np.float32)
    for h in range(HEADS):
        for cc in range(NCLS):
            meanw[3 * h + cc, cc] = 1.0 / 3.0
    ones3 = np.ones((NCLS, 1), np.float32)
    ones1 = np.ones((1, NCLS), np.float32)
    b1m = b1v.reshape(9, 1).copy()
    b2m = np.zeros((9, 1), np.float32)
    b2m[:NCLS, 0] = b2v

    idx_wr = np.stack([_wrap_chunked(streams[c], SCHUNK) for c in range(NCORE)])
    ov_wr = np.stack(
        [
            _wrap_chunked(np.repeat(ovidx[c].reshape(1, CN), 8, axis=0), DCHUNK)
            for c in range(NCORE)
        ]
    )

    def make_adrep(tab):
        out = []
        for c in range(NCORE):
            ad = np.zeros((3, RPAD), np.float32)
            ad[:, :CN] = tab[12:15, CN * c:CN * (c + 1)]
            for i, d in enumerate(ovdst[c]):
                ad[:, CN + i] = tab[12:15, CN * c + d]
            rep = np.zeros((24, RPAD), np.float32)
            for g in range(8):
                rep[3 * g:3 * g + 3, :] = ad
            out.append(rep)
        return out

    trace = bool(int(_os.environ.get("KERNEL_TRACE", "0")))
    stats = {}
    t0 = _time.time()

    ncA = _build_phase_a()
    in_maps = [
        {
            "xT": np.ascontiguousarray(xT[:, CN * c:CN * (c + 1)]),
            "w1": W1,
            "w1t": np.ascontiguousarray(W1.T),
            "attw1": attw1,
        }
        for c in range(NCORE)
    ]
    resA = _run(ncA, in_maps, trace=trace)
    stats["A_ns"] = resA.exec_time_ns
    tab1 = np.concatenate([resA.results[c]["tab"] for c in range(NCORE)], axis=1)
    padmask = np.zeros(NPAD, bool)
    for qq in range(NQ):
        padmask[QN * qq + QREAL:QN * (qq + 1)] = True
    tab1[9:12, padmask] = BIG_NEG

    ncB = _build_edge(final=False)
    adreps = make_adrep(tab1)
    in_maps = [
        {
            "tabf": tab1,
            "idxs": idx_wr[c],
            "ovidx": ov_wr[c],
            "adrep": adreps[c],
            "lhsn": lhsn,
            "lhsd": lhsd,
            "biasv": b1m,
            "w2t": np.ascontiguousarray(W2.T),
            "w2": W2,
            "attw2": attw2,
        }
        for c in range(NCORE)
    ]
    resB = _run(ncB, in_maps, trace=trace)
    stats["B_ns"] = resB.exec_time_ns
    tab2 = np.concatenate([resB.results[c]["tab2"] for c in range(NCORE)], axis=1)
    tab2[9:12, padmask] = BIG_NEG

    ncC = _build_edge(final=True)
    adreps = make_adrep(tab2)
    in_maps = [
        {
            "tabf": tab2,
            "idxs": idx_wr[c],
            "ovidx": ov_wr[c],
            "adrep": adreps[c],
            "lhsn": lhsn,
            "lhsd": lhsd,
            "biasv": b2m,
            "meanw": meanw,
            "ones3": ones3,
            "ones1": ones1,
        }
        for c in range(NCORE)
    ]
    resC = _run(ncC, in_maps, trace=trace)
    stats["C_ns"] = resC.exec_time_ns
    outT = np.concatenate([resC.results[c]["outp"] for c in range(NCORE)], axis=1)
    stats["wall_s"] = _time.time() - t0

    out = outT.T[_relabel(np.arange(N_NODES))]
    LAST_STATS.clear()
    LAST_STATS.update(stats)
    return np.ascontiguousarray(out, dtype=np.float32)



# revision 13
# speedup vs baseline: 1.0421x; 1.0421x over previous
"""GAT (2-layer, 3-head) forward on 8 Trainium2 NeuronCores.

Sharding: nodes split 8 ways; each core owns 12544 padded destination nodes
and all their incoming edges (1D graph partition per the spec hint). A
channel-major node table (h | a_src | a_dst, 15 ch) is replicated into SBUF
as 4 quarters x 2 copies across the 8 GPSIMD 16-partition groups; per-edge
features stream out via ap_gather with per-group index streams laid out in
dst-canonical order with K=8 slots per (dst, group) (A/B copy balancing;
rare per-(dst,quarter) overflow rows are folded back with a second small
gather). Edge softmax runs densely on DVE/ACT over the slot grid; the
per-node combine (numerator/denominator matmul, softmax normalization, next
layer) is fused into the slot loop with a 2-batch skew so PE/DVE/ACT overlap
the gathers. DMA issue is split between the SP and ACT hardware DGE rings so
semaphore-blocked scatters never block prefetches. Three NEFF launches:
(A) table build (x @ W1aug on PE), (B) edge layer 1 + layer-2 table build,
(C) edge layer 2 + head-mean + log_softmax. Tables are all-gathered between
launches through the host.
"""
import sys
import types

sys.path.insert(0, "/opt/trn_rl_repo")
import numpy as np

N_NODES = 100000
IN_DIM = 256
HID = 3
HEADS = 3
NCLS = 3
NEG = 0.2
EPS = 1e-16

NQ = 4
QREAL = 25000
QN = 25088
NPAD = NQ * QN          # 100352
NCORE = 8
CN = NPAD // NCORE      # 12544
K = 8
DCHUNK = 224
NCHUNK = CN // DCHUNK   # 56
RPAD = CN + 2 * DCHUNK  # 12992
GCHUNK = RPAD // DCHUNK  # 58 (56 main + 2 overflow)
SLOTS = RPAD * K
SCHUNK = DCHUNK * K     # 1792
BCOLS = 2 * DCHUNK      # 448 combine batch width (fits one PSUM bank)
NBATCH = NCHUNK // 2    # 28
OVN = 2 * DCHUNK        # 448 overflow rows
SENT = QREAL
OVSENT = OVN - 1        # fold-gather sentinel column inside ovpart
CH = 15
BIG_NEG = -30000.0
PREF = 3                # idxc prefetch distance (chunks)
SKEWB = 2               # combine batch skew

LAST_STATS = {}


def _install_ntff_hook_module():
    if "antenv.axon_hooks" in sys.modules:
        return
    mod = types.ModuleType("antenv.axon_hooks")
    state = {"hook": None, "tried": False}

    def set_axon_ntff_profile_hook(hook):
        state["hook"] = hook

    def get_axon_ntff_profile_hook():
        if state["hook"] is None and not state["tried"]:
            state["tried"] = True
            try:
                from trn_agent_boot.trn_boot import _ntff_profile_via_ctypes

                state["hook"] = _ntff_profile_via_ctypes("/opt/axon/libaxon_pjrt.so")
            except Exception:
                state["hook"] = None
        return state["hook"]

    mod.set_axon_ntff_profile_hook = set_axon_ntff_profile_hook
    mod.get_axon_ntff_profile_hook = get_axon_ntff_profile_hook
    sys.modules["antenv.axon_hooks"] = mod


_install_ntff_hook_module()

import concourse.bass as bass
import concourse.mybir as mybir
import concourse.tile as tile
from concourse.bass_utils import run_bass_kernel_spmd
from concourse.library_overlay import lower_extended_insts
from concourse import library_config

F32 = mybir.dt.float32
I16 = mybir.dt.int16
ALU = mybir.AluOpType
ACT = mybir.ActivationFunctionType


def _split_wide_waits(nc):
    """Walrus here caps sync-wait commands per instruction; hoist excess waits
    onto preceding same-engine nofuse NOPs (engines execute in order)."""
    for fn in nc.m.functions:
        for bb in fn.blocks:
            new_insts = []
            for inst in bb.instructions:
                keep = 0 if isinstance(inst, mybir.InstDrain) else 1
                si = inst.sync_info
                if si is not None and si.on_wait is not None and len(si.on_wait) > keep:
                    waits = list(si.on_wait)
                    head, rest = (waits[:-keep], waits[-keep:]) if keep else (waits, [])
                    while head:
                        chunk, head = head[:1], head[1:]
                        nop = mybir.InstNoOp(name=f"I-{nc.next_id()}", ins=[], outs=[])
                        nop.engine = inst.engine
                        nop.bass_nofuse = True
                        nop.sync_info = mybir.SyncInfo(on_wait=chunk, on_update=[])
                        nc.register_instruction(nop, overwrite=True)
                        new_insts.append(nop)
                    inst.sync_info = mybir.SyncInfo(
                        on_wait=rest, on_update=list(si.on_update or [])
                    )
                new_insts.append(inst)
            bb.instructions.clear()
            for i in new_insts:
                bb.add_instruction(i)


def _run(nc, in_maps, trace=False):
    lower_extended_insts(nc)
    _split_wide_waits(nc)
    return run_bass_kernel_spmd(nc, in_maps, core_ids=list(range(NCORE)), trace=trace)


# ---------------------------------------------------------------- launch A
def _build_phase_a():
    nc = bass.Bass("TRN2")
    xT_d = nc.dram_tensor("xT", [IN_DIM, CN], F32, kind="ExternalInput")
    w1aug_d = nc.dram_tensor("w1aug", [128, 2 * CH], F32, kind="ExternalInput")
    tab_d = nc.dram_tensor("tab", [CH, CN], F32, kind="ExternalOutput")

    with tile.TileContext(nc) as tc:
        with (
            tc.tile_pool(name="const", bufs=1) as cpool,
            tc.tile_pool(name="io", bufs=4) as iopool,
            tc.tile_pool(name="ps", bufs=4, space="PSUM") as pspool,
        ):
            w1aug = cpool.tile([128, 2 * CH], F32)
            nc.sync.dma_start(w1aug[:], w1aug_d[:])
            for c in range(NCHUNK):
                cols = slice(DCHUNK * c, DCHUNK * (c + 1))
                ps = pspool.tile([CH, DCHUNK], F32, tag="ps")
                for k in range(2):
                    xc = iopool.tile([128, DCHUNK], F32, tag=f"xc{k}")
                    eng = nc.scalar if k == 0 else nc.sync
                    eng.dma_start(xc[:], xT_d[128 * k:128 * (k + 1), cols])
                    nc.tensor.matmul(
                        out=ps[:],
                        lhsT=w1aug[:, CH * k:CH * (k + 1)],
                        rhs=xc[:],
                        start=(k == 0),
                        stop=(k == 1),
                    )
                ob = iopool.tile([CH, DCHUNK], F32, tag="ob")
                nc.vector.tensor_copy(out=ob[:], in_=ps[:])
                nc.scalar.dma_start(tab_d[:, cols], ob[:])
    return nc


# ---------------------------------------------------------------- launch B/C
def _build_edge(final):
    nc = bass.Bass("TRN2")
    tab_d = nc.dram_tensor("tabf", [CH, NPAD], F32, kind="ExternalInput")
    idx_d = nc.dram_tensor("idxs", [128, SLOTS // 16], I16, kind="ExternalInput")
    ov_d = nc.dram_tensor("ovidx", [128, CN // 16], I16, kind="ExternalInput")
    adrep_d = nc.dram_tensor("adrep", [24, RPAD], F32, kind="ExternalInput")
    lhsnd_d = nc.dram_tensor("lhsnd", [128, 18], F32, kind="ExternalInput")
    bias_d = nc.dram_tensor("biasv", [9, 1], F32, kind="ExternalInput")
    if final:
        mw_d = nc.dram_tensor("meanw10", [10, NCLS], F32, kind="ExternalInput")
        ones3_d = nc.dram_tensor("ones3", [NCLS, 1], F32, kind="ExternalInput")
        ones1_d = nc.dram_tensor("ones1", [1, NCLS], F32, kind="ExternalInput")
        out_d = nc.dram_tensor("outp", [NCLS, CN], F32, kind="ExternalOutput")
    else:
        w2_d = nc.dram_tensor("w2aug10", [10, CH], F32, kind="ExternalInput")
        tab2_d = nc.dram_tensor("tab2", [CH, CN], F32, kind="ExternalOutput")

    with tile.TileContext(nc) as tc:
        with (
            tc.tile_pool(name="big", bufs=1) as bigpool,
            tc.tile_pool(name="gp", bufs=4) as gpool,
            tc.tile_pool(name="wt", bufs=2) as wtpool,
            tc.tile_pool(name="idx", bufs=PREF + 2) as idxpool,
            tc.tile_pool(name="ad", bufs=2) as adpool,
            tc.tile_pool(name="cb", bufs=2) as cbpool,
            tc.tile_pool(name="ps", bufs=2, space="PSUM") as pspool,
            tc.tile_pool(name="psf", bufs=2, space="PSUM") as psfpool,
            tc.tile_pool(name="psg", bufs=1, space="PSUM") as psgpool,
        ):
            table = bigpool.tile([128, QN], F32)
            ovpart = bigpool.tile([128, OVN], F32)
            for g in range(8):
                q = g % 4
                eng = nc.sync if g % 2 == 0 else nc.scalar
                eng.dma_start(
                    table[16 * g:16 * g + CH, :], tab_d[:, QN * q:QN * (q + 1)]
                )
            lhsnd = bigpool.tile([128, 18], F32)
            nc.sync.dma_start(lhsnd[:], lhsnd_d[:])
            biasv = bigpool.tile([9, 1], F32)
            nc.sync.dma_start(biasv[:], bias_d[:])
            if final:
                meanw = bigpool.tile([10, NCLS], F32)
                ones3 = bigpool.tile([NCLS, 1], F32)
                ones1 = bigpool.tile([1, NCLS], F32)
                nc.sync.dma_start(meanw[:], mw_d[:])
                nc.sync.dma_start(ones3[:], ones3_d[:])
                nc.sync.dma_start(ones1[:], ones1_d[:])
            else:
                w2aug = bigpool.tile([10, CH], F32)
                nc.sync.dma_start(w2aug[:], w2_d[:])

            # rotating rings (manual; contents partially persistent)
            w9s = [
                bigpool.tile([128, SCHUNK], F32, name=f"w9_{i}") for i in range(3)
            ]
            for t in w9s:
                nc.vector.memset(t[:], 1.0)
            hags = [
                bigpool.tile([10, BCOLS], F32, name=f"hag_{i}") for i in range(3)
            ]
            onesrow = bigpool.tile([1, BCOLS], F32)
            nc.vector.memset(onesrow[:], 1.0)
            for t in hags:
                nc.sync.dma_start(t[9:10, :], onesrow[:])
            parts = [
                bigpool.tile([128, BCOLS], F32, name=f"part_{i}") for i in range(3)
            ]
            epsb = bigpool.tile([9, 1], F32)
            nc.vector.memset(epsb[:], EPS)

            tab_in = table[:].rearrange("p (n d) -> p n d", d=1)
            ov_in = ovpart[:].rearrange("p (n d) -> p n d", d=1)
            nc.gpsimd.load_library(library_config.ap_gather)

            def load_idxc(c):
                scol = slice(SCHUNK // 16 * c, SCHUNK // 16 * (c + 1))
                t = idxpool.tile([128, SCHUNK // 16], I16, tag="idxc")
                nc.scalar.dma_start(t[:], idx_d[:, scol])
                return t

            def load_a24(c):
                # one [24, BCOLS] tile per 2-chunk batch (c even)
                t = adpool.tile([24, BCOLS], F32, tag="a24")
                nc.scalar.dma_start(t[:], adrep_d[:, DCHUNK * c:DCHUNK * c + BCOLS])
                return t

            idxq = {}
            a24q = {}
            # chunk order: overflow chunks first so ovpart is ready early
            order = [NCHUNK, NCHUNK + 1] + list(range(NCHUNK))
            for i in range(min(PREF, len(order))):
                idxq[order[i]] = load_idxc(order[i])
            a24q[NCHUNK] = load_a24(NCHUNK)
            if order[2] == 0:
                a24q[0] = load_a24(0)

            def gather_chunk(pos, c):
                # prefetch idx stream / a_dst rows for later chunks
                if pos + PREF < len(order):
                    nx = order[pos + PREF]
                    idxq[nx] = load_idxc(nx)
                    if nx % 2 == 0 and nx not in a24q:
                        a24q[nx] = load_a24(nx)
                idxc = idxq.pop(c)
                g_t = gpool.tile([128, SCHUNK], F32, tag="g")
                nc.gpsimd.ap_gather(
                    out_ap=g_t[:].rearrange("p (n d) -> p n d", d=1),
                    in_ap=tab_in,
                    idxs_ap=idxc[:],
                    channels=128,
                    num_elems=QN,
                    d=1,
                    num_idxs=SCHUNK,
                )
                ceven = c - (c % 2)
                a24 = a24q[ceven]
                ahalf = a24[:, (c % 2) * DCHUNK:(c % 2) * DCHUNK + DCHUNK]
                wt = wtpool.tile([24, SCHUNK], F32, tag="wt")
                for g in range(8):
                    nc.sync.dma_start(
                        wt[3 * g:3 * g + 3, :], g_t[16 * g + 9:16 * g + 12, :]
                    )
                wt3 = wt[:].rearrange("p (n j) -> p n j", j=K)
                nc.vector.tensor_tensor(
                    out=wt3, in0=wt3,
                    in1=ahalf.to_broadcast([24, DCHUNK, K]), op=ALU.add,
                )
                nc.scalar.activation(out=wt[:], in_=wt[:], func=ACT.Lrelu, alpha=NEG)
                nc.scalar.activation(out=wt[:], in_=wt[:], func=ACT.Exp)
                w9 = w9s[pos % 3]
                for h in range(3):
                    nc.sync.dma_start(g_t[9 + h::16, :], wt[h::3, :])
                    for ch3 in range(3):
                        nc.sync.dma_start(w9[3 * h + ch3::16, :], wt[h::3, :])
                nc.vector.tensor_tensor(
                    out=g_t[:], in0=g_t[:], in1=w9[:], op=ALU.mult
                )
                if c >= NCHUNK:
                    dest = ovpart[:, DCHUNK * (c - NCHUNK):DCHUNK * (c - NCHUNK + 1)]
                else:
                    pb = parts[(c // 2) % 3]
                    dest = pb[:, (c % 2) * DCHUNK:(c % 2) * DCHUNK + DCHUNK]
                nc.vector.tensor_reduce(
                    out=dest,
                    in_=g_t[:].rearrange("p (n j) -> p n j", j=K),
                    axis=mybir.AxisListType.X,
                    op=ALU.add,
                )

            def combine_batch(b):
                cols = slice(BCOLS * b, BCOLS * (b + 1))
                pb = parts[b % 3]
                ovc = idxpool.tile([128, BCOLS // 16], I16, tag="ovc")
                nc.scalar.dma_start(
                    ovc[:], ov_d[:, BCOLS // 16 * b:BCOLS // 16 * (b + 1)]
                )
                foldt = cbpool.tile([128, BCOLS], F32, tag="fold")
                nc.gpsimd.ap_gather(
                    out_ap=foldt[:].rearrange("p (n d) -> p n d", d=1),
                    in_ap=ov_in,
                    idxs_ap=ovc[:],
                    channels=128,
                    num_elems=OVN,
                    d=1,
                    num_idxs=BCOLS,
                )
                ndn_ps = pspool.tile([9, BCOLS], F32, tag="ndn")
                ndd_ps = pspool.tile([9, BCOLS], F32, tag="ndd")
                nc.tensor.matmul(
                    out=ndn_ps[:], lhsT=lhsnd[:, 0:9], rhs=pb[:],
                    start=True, stop=False,
                )
                nc.tensor.matmul(
                    out=ndn_ps[:], lhsT=lhsnd[:, 0:9], rhs=foldt[:],
                    start=False, stop=True,
                )
                nc.tensor.matmul(
                    out=ndd_ps[:], lhsT=lhsnd[:, 9:18], rhs=pb[:],
                    start=True, stop=False,
                )
                nc.tensor.matmul(
                    out=ndd_ps[:], lhsT=lhsnd[:, 9:18], rhs=foldt[:],
                    start=False, stop=True,
                )
                ndd = cbpool.tile([9, BCOLS], F32, tag="ndd_sb")
                nc.scalar.activation(
                    out=ndd[:], in_=ndd_ps[:], func=ACT.Identity, bias=epsb[:]
                )
                rden = cbpool.tile([9, BCOLS], F32, tag="rden")
                nc.vector.reciprocal(out=rden[:], in_=ndd[:])
                hag = hags[b % 3]
                nc.vector.tensor_tensor(
                    out=hag[0:9, :], in0=ndn_ps[:], in1=rden[:], op=ALU.mult
                )
                if not final:
                    nc.vector.tensor_tensor(
                        out=hag[0:9, :], in0=hag[0:9, :],
                        in1=biasv[:].to_broadcast([9, BCOLS]), op=ALU.add,
                    )
                    t1 = cbpool.tile([9, BCOLS], F32, tag="t1")
                    nc.vector.tensor_scalar_min(out=t1[:], in0=hag[0:9, :], scalar1=0.0)
                    nc.scalar.activation(out=t1[:], in_=t1[:], func=ACT.Exp)
                    # rows 0..8 <- relu(h) + exp(min(h,0)); the elu "-1" and the
                    # attention projections are folded into w2aug row 9 (host)
                    nc.vector.scalar_tensor_tensor(
                        out=hag[0:9, :], in0=hag[0:9, :], scalar=0.0, in1=t1[:],
                        op0=ALU.max, op1=ALU.add,
                    )
                    t2ps = psfpool.tile([CH, BCOLS], F32, tag="t2")
                    nc.tensor.matmul(
                        out=t2ps[:], lhsT=w2aug[:], rhs=hag[:], start=True, stop=True
                    )
                    t2sb = cbpool.tile([CH, BCOLS], F32, tag="t2sb")
                    nc.vector.tensor_copy(out=t2sb[:], in_=t2ps[:])
                    nc.scalar.dma_start(tab2_d[:, cols], t2sb[:])
                else:
                    zps = psfpool.tile([NCLS, BCOLS], F32, tag="z")
                    nc.tensor.matmul(
                        out=zps[:], lhsT=meanw[:], rhs=hag[:], start=True, stop=True
                    )
                    ez = cbpool.tile([NCLS, BCOLS], F32, tag="ez")
                    nc.scalar.activation(out=ez[:], in_=zps[:], func=ACT.Exp)
                    zsb = cbpool.tile([NCLS, BCOLS], F32, tag="zsb")
                    nc.vector.tensor_copy(out=zsb[:], in_=zps[:])
                    sps = psgpool.tile([1, BCOLS], F32, tag="s")
                    nc.tensor.matmul(
                        out=sps[:], lhsT=ones3[:], rhs=ez[:], start=True, stop=True
                    )
                    s = cbpool.tile([1, BCOLS], F32, tag="sl")
                    nc.scalar.activation(out=s[:], in_=sps[:], func=ACT.Ln)
                    l3ps = psgpool.tile([NCLS, BCOLS], F32, tag="l3")
                    nc.tensor.matmul(
                        out=l3ps[:], lhsT=ones1[:], rhs=s[:], start=True, stop=True
                    )
                    zm = cbpool.tile([NCLS, BCOLS], F32, tag="zm")
                    nc.vector.tensor_tensor(
                        out=zm[:], in0=zsb[:], in1=l3ps[:], op=ALU.subtract
                    )
                    nc.scalar.dma_start(out_d[:, cols], zm[:])

            for pos, c in enumerate(order):
                gather_chunk(pos, c)
                if c < NCHUNK and c % 2 == 1:
                    b = c // 2
                    if b >= SKEWB:
                        combine_batch(b - SKEWB)
            for b in range(NBATCH - SKEWB, NBATCH):
                combine_batch(b)
    return nc


# ---------------------------------------------------------------- host side
def _relabel(n):
    q = n // QREAL
    return q * QN + n % QREAL


def _wrap_chunked(stream, chunk):
    """[G, S] streams -> [16G, S//16] ap_gather idx layout, wrapped per chunk."""
    g, s = stream.shape
    nch = s // chunk
    w = stream.reshape(g, nch, chunk // 16, 16)
    w = w.transpose(0, 3, 1, 2)
    return np.ascontiguousarray(w.reshape(g * 16, s // 16))


def _pack_edges(src, dst):
    srcN = _relabel(src.astype(np.int64))
    dstN = _relabel(dst.astype(np.int64))
    core = dstN // CN
    dloc = dstN % CN
    q = srcN // QN
    sloc = (srcN % QN).astype(np.int16)

    key = (core * CN + dloc) * 4 + q
    order = np.argsort(key, kind="stable")
    ks = key[order]
    grp_start = np.r_[0, np.flatnonzero(np.diff(ks)) + 1]
    sizes = np.diff(np.r_[grp_start, len(ks)])
    rank = np.arange(len(ks)) - np.repeat(grp_start, sizes)

    co, dl, qo, sl = core[order], dloc[order], q[order], sloc[order]

    streams = np.full((NCORE, 8, SLOTS), SENT, dtype=np.int16)
    ovidx = np.full((NCORE, CN), OVSENT, dtype=np.int16)
    ovdst = [[] for _ in range(NCORE)]

    main = rank < 16
    gmain = qo[main] + 4 * (rank[main] & 1)
    pos = dl[main] * K + (rank[main] >> 1)
    streams[co[main], gmain, pos] = sl[main]

    for i in np.flatnonzero(~main):
        c, d, qq, s_, r = co[i], dl[i], qo[i], sl[i], rank[i]
        if ovidx[c, d] == OVSENT:
            row = len(ovdst[c])
            assert row < OVN - 1, "overflow area exhausted"
            ovidx[c, d] = row
            ovdst[c].append(int(d))
        rr = r - 16
        assert rr < 16, "per-(dst,quarter) capacity exceeded"
        g = qq + 4 * (rr & 1)
        streams[c, g, (CN + int(ovidx[c, d])) * K + (rr >> 1)] = s_
    return streams, ovidx, ovdst


def kernel(x, edge_index, W1, att_src1, att_dst1, b1, W2, att_src2, att_dst2, b2):
    import os as _os
    import time as _time

    x = np.asarray(x, np.float32)
    W1 = np.asarray(W1, np.float32)
    W2 = np.asarray(W2, np.float32)
    b1v = np.asarray(b1, np.float32)
    b2v = np.asarray(b2, np.float32)

    loops = np.arange(N_NODES, dtype=np.int64)
    src = np.concatenate([np.asarray(edge_index[0], np.int64), loops])
    dst = np.concatenate([np.asarray(edge_index[1], np.int64), loops])
    streams, ovidx, ovdst = _pack_edges(src, dst)

    xP = np.zeros((NPAD, IN_DIM), np.float32)
    xP[_relabel(np.arange(N_NODES))] = x
    xT = np.ascontiguousarray(xP.T)

    def attw(att_s, att_d):
        a = np.zeros((HEADS * HID, 6), np.float32)
        for h in range(HEADS):
            for cc in range(3):
                a[3 * h + cc, h] = np.asarray(att_s, np.float32)[h, cc]
                a[3 * h + cc, 3 + h] = np.asarray(att_d, np.float32)[h, cc]
        return a

    attw1 = attw(att_src1, att_dst1)
    attw2 = attw(att_src2, att_dst2)

    # layer-1 augmented weights: [x (256)] @ [W1 | W1@attw1] -> 15ch table
    w1aug = np.zeros((128, 2 * CH), np.float32)
    v1 = W1 @ attw1  # [256, 6]
    for k in range(2):
        w1aug[:, CH * k:CH * k + 9] = W1[128 * k:128 * (k + 1), :]
        w1aug[:, CH * k + 9:CH * k + 15] = v1[128 * k:128 * (k + 1), :]

    # layer-2 augmented weights, with elu "-1" shift folded into row 9
    w2aug10 = np.zeros((10, CH), np.float32)
    w2aug10[:9, 0:9] = W2
    w2aug10[:9, 9:15] = W2 @ attw2
    w2aug10[9, :] = -w2aug10[:9, :].sum(axis=0)

    lhsnd = np.zeros((128, 18), np.float32)
    for p in range(128):
        j = p % 16
        if j < 9:
            lhsnd[p, j] = 1.0
        elif j < 12:
            for cc in range(3):
                lhsnd[p, 9 + 3 * (j - 9) + cc] = 1.0
    meanw10 = np.zeros((10, NCLS), np.float32)
    for h in range(HEADS):
        for cc in range(NCLS):
            meanw10[3 * h + cc, cc] = 1.0 / 3.0
    meanw10[9, :] = b2v
    ones3 = np.ones((NCLS, 1), np.float32)
    ones1 = np.ones((1, NCLS), np.float32)
    b1m = b1v.reshape(9, 1).copy()
    b0m = np.zeros((9, 1), np.float32)

    idx_wr = np.stack([_wrap_chunked(streams[c], SCHUNK) for c in range(NCORE)])
    ov_wr = np.stack(
        [
            _wrap_chunked(np.repeat(ovidx[c].reshape(1, CN), 8, axis=0), BCOLS)
            for c in range(NCORE)
        ]
    )

    def make_adrep(tab):
        out = []
        for c in range(NCORE):
            ad = np.zeros((3, RPAD), np.float32)
            ad[:, :CN] = tab[12:15, CN * c:CN * (c + 1)]
            for i, d in enumerate(ovdst[c]):
                ad[:, CN + i] = tab[12:15, CN * c + d]
            rep = np.zeros((24, RPAD), np.float32)
            for g in range(8):
                rep[3 * g:3 * g + 3, :] = ad
            out.append(rep)
        return out

    trace = bool(int(_os.environ.get("KERNEL_TRACE", "0")))
    stats = {}
    t0 = _time.time()

    ncA = _build_phase_a()
    in_maps = [
        {
            "xT": np.ascontiguousarray(xT[:, CN * c:CN * (c + 1)]),
            "w1aug": w1aug,
        }
        for c in range(NCORE)
    ]
    resA = _run(ncA, in_maps, trace=trace)
    stats["A_ns"] = resA.exec_time_ns
    tab1 = np.concatenate([resA.results[c]["tab"] for c in range(NCORE)], axis=1)
    padmask = np.zeros(NPAD, bool)
    for qq in range(NQ):
        padmask[QN * qq + QREAL:QN * (qq + 1)] = True
    tab1[9:12, padmask] = BIG_NEG

    ncB = _build_edge(final=False)
    adreps = make_adrep(tab1)
    in_maps = [
        {
            "tabf": tab1,
            "idxs": idx_wr[c],
            "ovidx": ov_wr[c],
            "adrep": adreps[c],
            "lhsnd": lhsnd,
            "biasv": b1m,
            "w2aug10": w2aug10,
        }
        for c in range(NCORE)
    ]
    resB = _run(ncB, in_maps, trace=trace)
    stats["B_ns"] = resB.exec_time_ns
    tab2 = np.concatenate([resB.results[c]["tab2"] for c in range(NCORE)], axis=1)
    tab2[9:12, padmask] = BIG_NEG

    ncC = _build_edge(final=True)
    adreps = make_adrep(tab2)
    in_maps = [
        {
            "tabf": tab2,
            "idxs": idx_wr[c],
            "ovidx": ov_wr[c],
            "adrep": adreps[c],
            "lhsnd": lhsnd,
            "biasv": b0m,
            "meanw10": meanw10,
            "ones3": ones3,
            "ones1": ones1,
        }
        for c in range(NCORE)
    ]
    resC = _run(ncC, in_maps, trace=trace)
    stats["C_ns"] = resC.exec_time_ns
    outT = np.concatenate([resC.results[c]["outp"] for c in range(NCORE)], axis=1)
    stats["wall_s"] = _time.time() - t0

    out = outT.T[_relabel(np.arange(N_NODES))]
    LAST_STATS.clear()
    LAST_STATS.update(stats)
    return np.ascontiguousarray(out, dtype=np.float32)


# revision 16
# speedup vs baseline: 1.6057x; 1.5409x over previous
"""GAT (2-layer, 3-head) forward on 8 Trainium2 NeuronCores.

Sharding: nodes split 8 ways; each core owns 12544 padded destination nodes
and all their incoming edges (1D graph partition per the spec hint). A
channel-major node table (h | a_src | a_dst, 15 ch) is replicated into SBUF
as 4 quarters x 2 copies across the 8 GPSIMD 16-partition groups; per-edge
features stream out via ap_gather with per-group index streams in
dst-canonical order. Destinations are sorted by their max per-quarter
in-degree and packed into 224-dst chunks with a per-chunk slot width K
(capacity 2K per (dst, quarter); A/B copy balancing; rare overflow edges go
to dedicated overflow rows processed as two extra K=8 chunks). Edge softmax
weights run densely on ACT (Lrelu/Exp) over the slot grid; weighted messages
reduce per dst on DVE; per-2-chunk partial tables [128, 448] stream to DRAM.
The cheap O(N) combine (overflow fold, group sum, softmax normalization,
bias/ELU, W2 projection, head mean + log_softmax) runs on the host between
launches. Three NEFF launches: (A) x @ W1aug table build on PE, then one
shared edge program run twice (layers 1 and 2)."""
import sys
import types

sys.path.insert(0, "/opt/trn_rl_repo")
import numpy as np

N_NODES = 100000
IN_DIM = 256
HID = 3
HEADS = 3
NCLS = 3
NEG = 0.2
EPS = 1e-16

NQ = 4
QREAL = 25000
QN = 25088
NPAD = NQ * QN          # 100352
NCORE = 8
CN = NPAD // NCORE      # 12544
KMAX = 8
DCHUNK = 224
NCHUNK = CN // DCHUNK   # 56
OVN = 2 * DCHUNK        # 448 overflow rows
RPAD = CN + OVN
SENT = QREAL
CH = 15
BIG_NEG = -30000.0
PREF = 3

LAST_STATS = {}


def _install_ntff_hook_module():
    if "antenv.axon_hooks" in sys.modules:
        return
    mod = types.ModuleType("antenv.axon_hooks")
    state = {"hook": None, "tried": False}

    def set_axon_ntff_profile_hook(hook):
        state["hook"] = hook

    def get_axon_ntff_profile_hook():
        if state["hook"] is None and not state["tried"]:
            state["tried"] = True
            try:
                from trn_agent_boot.trn_boot import _ntff_profile_via_ctypes

                state["hook"] = _ntff_profile_via_ctypes("/opt/axon/libaxon_pjrt.so")
            except Exception:
                state["hook"] = None
        return state["hook"]

    mod.set_axon_ntff_profile_hook = set_axon_ntff_profile_hook
    mod.get_axon_ntff_profile_hook = get_axon_ntff_profile_hook
    sys.modules["antenv.axon_hooks"] = mod


_install_ntff_hook_module()

import concourse.bass as bass
import concourse.mybir as mybir
import concourse.tile as tile
from concourse.bass_utils import run_bass_kernel_spmd
from concourse.library_overlay import lower_extended_insts
from concourse import library_config

F32 = mybir.dt.float32
I16 = mybir.dt.int16
ALU = mybir.AluOpType
ACT = mybir.ActivationFunctionType


def _split_wide_waits(nc):
    """Walrus here caps sync-wait commands per instruction; hoist excess waits
    onto preceding same-engine nofuse NOPs (engines execute in order)."""
    for fn in nc.m.functions:
        for bb in fn.blocks:
            new_insts = []
            for inst in bb.instructions:
                keep = 0 if isinstance(inst, mybir.InstDrain) else 1
                si = inst.sync_info
                if si is not None and si.on_wait is not None and len(si.on_wait) > keep:
                    waits = list(si.on_wait)
                    head, rest = (waits[:-keep], waits[-keep:]) if keep else (waits, [])
                    while head:
                        chunk, head = head[:1], head[1:]
                        nop = mybir.InstNoOp(name=f"I-{nc.next_id()}", ins=[], outs=[])
                        nop.engine = inst.engine
                        nop.bass_nofuse = True
                        nop.sync_info = mybir.SyncInfo(on_wait=chunk, on_update=[])
                        nc.register_instruction(nop, overwrite=True)
                        new_insts.append(nop)
                    inst.sync_info = mybir.SyncInfo(
                        on_wait=rest, on_update=list(si.on_update or [])
                    )
                new_insts.append(inst)
            bb.instructions.clear()
            for i in new_insts:
                bb.add_instruction(i)


def _run(nc, in_maps, trace=False):
    lower_extended_insts(nc)
    _split_wide_waits(nc)
    return run_bass_kernel_spmd(nc, in_maps, core_ids=list(range(NCORE)), trace=trace)


# ---------------------------------------------------------------- launch A
def _build_phase_a():
    nc = bass.Bass("TRN2")
    xT_d = nc.dram_tensor("xT", [IN_DIM, CN], F32, kind="ExternalInput")
    w1aug_d = nc.dram_tensor("w1aug", [128, 2 * CH], F32, kind="ExternalInput")
    tab_d = nc.dram_tensor("tab", [CH, CN], F32, kind="ExternalOutput")

    with tile.TileContext(nc) as tc:
        with (
            tc.tile_pool(name="const", bufs=1) as cpool,
            tc.tile_pool(name="io", bufs=4) as iopool,
            tc.tile_pool(name="ps", bufs=4, space="PSUM") as pspool,
        ):
            w1aug = cpool.tile([128, 2 * CH], F32)
            nc.sync.dma_start(w1aug[:], w1aug_d[:])
            for c in range(NCHUNK):
                cols = slice(DCHUNK * c, DCHUNK * (c + 1))
                ps = pspool.tile([CH, DCHUNK], F32, tag="ps")
                for k in range(2):
                    xc = iopool.tile([128, DCHUNK], F32, tag=f"xc{k}")
                    eng = nc.scalar if k == 0 else nc.sync
                    eng.dma_start(xc[:], xT_d[128 * k:128 * (k + 1), cols])
                    nc.tensor.matmul(
                        out=ps[:],
                        lhsT=w1aug[:, CH * k:CH * (k + 1)],
                        rhs=xc[:],
                        start=(k == 0),
                        stop=(k == 1),
                    )
                ob = iopool.tile([CH, DCHUNK], F32, tag="ob")
                nc.vector.tensor_copy(out=ob[:], in_=ps[:])
                nc.scalar.dma_start(tab_d[:, cols], ob[:])
    return nc


# ---------------------------------------------------------------- edge launch
def _build_edge(Ks, scol_off, stot):
    """One edge-layer pass: gather + edge softmax weights + weighted reduce.
    Ks: per-chunk slot width (58 entries, last two are the overflow chunks).
    scol_off: per-chunk offset into the wrapped idx stream (units of 16 idxs).
    stot: total per-group slots (idx stream length)."""
    nc = bass.Bass("TRN2")
    tab_d = nc.dram_tensor("tabf", [CH, NPAD], F32, kind="ExternalInput")
    idx_d = nc.dram_tensor("idxs", [128, stot // 16], I16, kind="ExternalInput")
    adrep_d = nc.dram_tensor("adrep", [24, RPAD], F32, kind="ExternalInput")
    parts_d = nc.dram_tensor("parts", [128, RPAD], F32, kind="ExternalOutput")

    nch = len(Ks)
    SMAX = DCHUNK * KMAX
    with tile.TileContext(nc) as tc:
        with (
            tc.tile_pool(name="big", bufs=1) as bigpool,
            tc.tile_pool(name="gp", bufs=5) as gpool,
            tc.tile_pool(name="wt", bufs=3) as wtpool,
            tc.tile_pool(name="idx", bufs=PREF + 2) as idxpool,
            tc.tile_pool(name="ad", bufs=2) as adpool,
            tc.tile_pool(name="pb", bufs=3) as pbpool,
        ):
            table = bigpool.tile([128, QN], F32)
            for g in range(8):
                q = g % 4
                eng = nc.sync if g % 2 == 0 else nc.scalar
                eng.dma_start(
                    table[16 * g:16 * g + CH, :], tab_d[:, QN * q:QN * (q + 1)]
                )
            w9s = [
                bigpool.tile([128, SMAX], F32, name=f"w9_{i}") for i in range(3)
            ]
            for t in w9s:
                nc.vector.memset(t[:], 1.0)

            tab_in = table[:].rearrange("p (n d) -> p n d", d=1)
            nc.gpsimd.load_library(library_config.ap_gather)

            def load_idxc(c):
                w = DCHUNK * Ks[c] // 16
                t = idxpool.tile([128, SMAX // 16], I16, tag="idxc")
                nc.scalar.dma_start(
                    t[:, :w], idx_d[:, scol_off[c]:scol_off[c] + w]
                )
                return t

            def load_a24(c):
                t = adpool.tile([24, 2 * DCHUNK], F32, tag="a24")
                nc.scalar.dma_start(
                    t[:], adrep_d[:, DCHUNK * c:DCHUNK * (c + 2)]
                )
                return t

            idxq = {c: load_idxc(c) for c in range(min(PREF, nch))}
            a24q = {c: load_a24(c) for c in range(0, min(PREF + 1, nch), 2)}
            pbq = {}

            for c in range(nch):
                K = Ks[c]
                S = DCHUNK * K
                if c + PREF < nch:
                    idxq[c + PREF] = load_idxc(c + PREF)
                    if (c + PREF) % 2 == 0:
                        a24q[c + PREF] = load_a24(c + PREF)
                idxc = idxq.pop(c)
                g_t = gpool.tile([128, SMAX], F32, tag="g")
                nc.gpsimd.ap_gather(
                    out_ap=g_t[:, :S].rearrange("p (n d) -> p n d", d=1),
                    in_ap=tab_in,
                    idxs_ap=idxc[:, :S // 16],
                    channels=128,
                    num_elems=QN,
                    d=1,
                    num_idxs=S,
                )
                a24 = a24q[c - (c % 2)]
                ahalf = a24[:, (c % 2) * DCHUNK:(c % 2 + 1) * DCHUNK]
                wt = wtpool.tile([24, SMAX], F32, tag="wt")
                for g in range(8):
                    nc.sync.dma_start(
                        wt[3 * g:3 * g + 3, :S], g_t[16 * g + 9:16 * g + 12, :S]
                    )
                wt3 = wt[:, :S].rearrange("p (n j) -> p n j", j=K)
                nc.vector.tensor_tensor(
                    out=wt3, in0=wt3,
                    in1=ahalf.to_broadcast([24, DCHUNK, K]), op=ALU.add,
                )
                nc.scalar.activation(
                    out=wt[:, :S], in_=wt[:, :S], func=ACT.Lrelu, alpha=NEG
                )
                nc.scalar.activation(out=wt[:, :S], in_=wt[:, :S], func=ACT.Exp)
                w9 = w9s[c % 3]
                for h in range(3):
                    nc.sync.dma_start(g_t[9 + h::16, :S], wt[h::3, :S])
                    for ch3 in range(3):
                        nc.sync.dma_start(w9[3 * h + ch3::16, :S], wt[h::3, :S])
                nc.vector.tensor_tensor(
                    out=g_t[:, :S], in0=g_t[:, :S], in1=w9[:, :S], op=ALU.mult
                )
                if c % 2 == 0:
                    pbq[c // 2] = pbpool.tile([128, 2 * DCHUNK], F32, name="pb", tag="pb")
                pb = pbq[c // 2]
                nc.vector.tensor_reduce(
                    out=pb[:, (c % 2) * DCHUNK:(c % 2 + 1) * DCHUNK],
                    in_=g_t[:, :S].rearrange("p (n j) -> p n j", j=K),
                    axis=mybir.AxisListType.X,
                    op=ALU.add,
                )
                if c % 2 == 1:
                    b = c // 2
                    nc.scalar.dma_start(
                        parts_d[:, 2 * DCHUNK * b:2 * DCHUNK * (b + 1)],
                        pbq.pop(b)[:],
                    )
    return nc


# ---------------------------------------------------------------- host side
def _relabel(n):
    q = n // QREAL
    return q * QN + n % QREAL


def _wrap16(stream):
    """[8, S] group streams -> [128, S//16] ap_gather idx layout."""
    g, s = stream.shape
    w = stream.reshape(g, s // 16, 16).transpose(0, 2, 1)
    return np.ascontiguousarray(w.reshape(g * 16, s // 16))


def _pack_edges(src, dst):
    srcN = _relabel(src.astype(np.int64))
    dstN = _relabel(dst.astype(np.int64))
    core = dstN // CN
    dloc = dstN % CN
    q = srcN // QN
    sloc = (srcN % QN).astype(np.int16)

    cnt = np.zeros((NCORE, CN, 4), np.int64)
    np.add.at(cnt, (core, dloc, q), 1)
    maxq = cnt.max(axis=2)

    orders = []
    Ks_all = []
    for c in range(NCORE):
        order = np.argsort(-maxq[c], kind="stable")
        m = maxq[c][order]
        Ks = []
        for b in range(NCHUNK):
            mm = m[b * DCHUNK:(b + 1) * DCHUNK].max()
            Ks.append(min(KMAX, max(1, int(-(-int(mm) // 2)))))
        orders.append(order)
        Ks_all.append(Ks + [KMAX, KMAX])
    # shared chunk schedule across cores (program is shared): use per-chunk max
    Ks = [max(Ks_all[c][i] for c in range(NCORE)) for i in range(NCHUNK + 2)]
    # sorted position of each dst
    spos = np.empty((NCORE, CN), np.int64)
    for c in range(NCORE):
        spos[c][orders[c]] = np.arange(CN)

    karr = np.array(Ks[:NCHUNK], np.int64)
    base = np.concatenate([[0], np.cumsum(DCHUNK * karr)])  # slot base per chunk
    stot = int(base[-1]) + OVN * KMAX
    ovbase = int(base[-1])

    # slot offset for each sorted dst position
    pos_chunk = np.arange(CN) // DCHUNK
    slot0 = base[pos_chunk] + (np.arange(CN) % DCHUNK) * karr[pos_chunk]
    cap = 2 * karr[pos_chunk]  # capacity per (dst, q) at sorted position

    key = (core * CN + dloc) * 4 + q
    order = np.argsort(key, kind="stable")
    ks = key[order]
    grp_start = np.r_[0, np.flatnonzero(np.diff(ks)) + 1]
    sizes = np.diff(np.r_[grp_start, len(ks)])
    rank = np.arange(len(ks)) - np.repeat(grp_start, sizes)

    co, dl, qo, sl = core[order], dloc[order], q[order], sloc[order]
    sp = spos[co, dl]
    scap = cap[sp]
    ssl0 = slot0[sp]
    kk = karr[sp // DCHUNK]

    streams = np.full((NCORE, 8, stot), SENT, dtype=np.int16)
    ovidx = np.full((NCORE, CN), OVN - 1, dtype=np.int16)
    ovdst = [[] for _ in range(NCORE)]

    main = rank < scap
    gmain = qo[main] + 4 * (rank[main] & 1)
    pos = ssl0[main] + (rank[main] >> 1)
    streams[co[main], gmain, pos] = sl[main]

    for i in np.flatnonzero(~main):
        c, s_p, qq, s_, r = co[i], int(sp[i]), qo[i], sl[i], rank[i]
        if ovidx[c, s_p] == OVN - 1:
            row = len(ovdst[c])
            assert row < OVN - 1, "overflow area exhausted"
            ovidx[c, s_p] = row
            ovdst[c].append(s_p)
        rr = r - scap[i]
        assert rr < 16, "overflow capacity exceeded"
        g = qq + 4 * (rr & 1)
        streams[c, g, ovbase + int(ovidx[c, s_p]) * KMAX + (rr >> 1)] = s_
    return streams, ovidx, ovdst, orders, Ks, base, stot


def kernel(x, edge_index, W1, att_src1, att_dst1, b1, W2, att_src2, att_dst2, b2):
    import os as _os
    import time as _time

    x = np.asarray(x, np.float32)
    W1 = np.asarray(W1, np.float32)
    W2 = np.asarray(W2, np.float32)
    b1v = np.asarray(b1, np.float32)
    b2v = np.asarray(b2, np.float32)

    loops = np.arange(N_NODES, dtype=np.int64)
    src = np.concatenate([np.asarray(edge_index[0], np.int64), loops])
    dst = np.concatenate([np.asarray(edge_index[1], np.int64), loops])
    streams, ovidx, ovdst, orders, Ks, base, stot = _pack_edges(src, dst)

    xP = np.zeros((NPAD, IN_DIM), np.float32)
    xP[_relabel(np.arange(N_NODES))] = x
    xT = np.ascontiguousarray(xP.T)

    def attw(att_s, att_d):
        a = np.zeros((HEADS * HID, 6), np.float32)
        for h in range(HEADS):
            for cc in range(3):
                a[3 * h + cc, h] = np.asarray(att_s, np.float32)[h, cc]
                a[3 * h + cc, 3 + h] = np.asarray(att_d, np.float32)[h, cc]
        return a

    attw1 = attw(att_src1, att_dst1)
    attw2 = attw(att_src2, att_dst2)

    w1aug = np.zeros((128, 2 * CH), np.float32)
    v1 = W1 @ attw1
    for k in range(2):
        w1aug[:, CH * k:CH * k + 9] = W1[128 * k:128 * (k + 1), :]
        w1aug[:, CH * k + 9:CH * k + 15] = v1[128 * k:128 * (k + 1), :]
    w2aug = np.concatenate([W2, W2 @ attw2], axis=1)  # [9, 15]

    # wrapped idx stream: per-chunk 16-wrap, concatenated
    kall = Ks[:NCHUNK] + [KMAX, KMAX]
    scol_off = []
    off = 0
    for K in kall:
        scol_off.append(off)
        off += DCHUNK * K // 16
    idx_wr = []
    for c in range(NCORE):
        blocks = []
        for i, K in enumerate(kall):
            lo = int(base[i]) if i < NCHUNK else int(base[-1]) + (i - NCHUNK) * DCHUNK * KMAX
            blocks.append(_wrap16(streams[c][:, lo:lo + DCHUNK * K]))
        idx_wr.append(np.concatenate(blocks, axis=1))

    def make_adrep(tab):
        out = []
        for c in range(NCORE):
            srt = orders[c]
            ad = np.zeros((3, RPAD), np.float32)
            ad[:, :CN] = tab[12:15, CN * c + srt]
            for i, d in enumerate(ovdst[c]):
                ad[:, CN + i] = tab[12:15, CN * c + srt[d]]
            rep = np.zeros((24, RPAD), np.float32)
            for g in range(8):
                rep[3 * g:3 * g + 3, :] = ad
            out.append(rep)
        return out

    padmask = np.zeros(NPAD, bool)
    for qq in range(NQ):
        padmask[QN * qq + QREAL:QN * (qq + 1)] = True

    def combine(parts_list):
        """Fold overflow + group-sum -> per-core num[9, CN], den[3, CN]
        in sorted dst order."""
        nums, dens = [], []
        for c in range(NCORE):
            P = parts_list[c].reshape(8, 16, RPAD)
            num = P[:, 0:9, :].sum(axis=0)
            den = P[:, 9:12, :].sum(axis=0)
            ovx = ovidx[c]
            num[:, :CN] += num[:, CN:][:, ovx]
            den[:, :CN] += den[:, CN:][:, ovx]
            nums.append(num[:, :CN])
            dens.append(den[:, :CN])
        return nums, dens

    trace = bool(int(_os.environ.get("KERNEL_TRACE", "0")))
    stats = {}
    t0 = _time.time()

    ncA = _build_phase_a()
    in_maps = [
        {
            "xT": np.ascontiguousarray(xT[:, CN * c:CN * (c + 1)]),
            "w1aug": w1aug,
        }
        for c in range(NCORE)
    ]
    resA = _run(ncA, in_maps, trace=trace)
    stats["A_ns"] = resA.exec_time_ns
    tab1 = np.concatenate([resA.results[c]["tab"] for c in range(NCORE)], axis=1)
    tab1[9:12, padmask] = BIG_NEG

    ncE = _build_edge(kall, scol_off, stot)

    def run_edge(tab, tag):
        adreps = make_adrep(tab)
        in_maps = [
            {"tabf": tab, "idxs": idx_wr[c], "adrep": adreps[c]}
            for c in range(NCORE)
        ]
        res = _run(ncE, in_maps, trace=trace)
        stats[tag] = res.exec_time_ns
        return [res.results[c]["parts"] for c in range(NCORE)]

    nums, dens = combine(run_edge(tab1, "B_ns"))
    tab2 = np.full((CH, NPAD), 0.0, np.float32)
    for c in range(NCORE):
        hag = nums[c] / (dens[c].repeat(3, axis=0) + EPS) + b1v[:, None]
        v = np.maximum(hag, 0) + np.exp(np.minimum(hag, 0)) - 1.0
        t2 = w2aug.T @ v  # [15, CN] sorted order
        tab2[:, CN * c + orders[c]] = t2
    tab2[9:12, padmask] = BIG_NEG

    nums, dens = combine(run_edge(tab2, "C_ns"))
    out = np.zeros((N_NODES, NCLS), np.float32)
    outP = np.zeros((NPAD, NCLS), np.float32)
    for c in range(NCORE):
        hag = nums[c] / (dens[c].repeat(3, axis=0) + EPS)
        z = hag.reshape(3, 3, CN).mean(axis=0) + b2v[:, None]  # [3, CN]
        z = z - np.log(np.exp(z).sum(axis=0, keepdims=True))
        outP[CN * c + orders[c]] = z.T
    out = outP[_relabel(np.arange(N_NODES))]
    stats["wall_s"] = _time.time() - t0

    LAST_STATS.clear()
    LAST_STATS.update(stats)
    return np.ascontiguousarray(out, dtype=np.float32)
